# revision 1
# baseline (speedup 1.0000x reference)
"""Trainium2 Bass kernel for nn_BinarizedCifar10MLP.

Strategy: data-parallel over the batch (8192/8 = 1024 rows per core), with
feature-major ("transposed") activation layout [features, batch] on device so
no on-device transposes are needed anywhere.  BatchNorm batch statistics are
all-reduced across the 8 cores (3 tiny AllReduces of [128,64] fp32).

Precision scheme (reference is fp32):
  - L1 (x @ sign(W1).T): x is split losslessly on the host into fp16 hi + lo
    pieces (x == hi + lo exactly); each piece is matmul'd against sign(W1) in
    fp16 at full PE rate and accumulated in the same fp32 PSUM group.  Since
    sign(W1) in fp16 is exact and fp16 products vs +-1 are exact, the result
    carries only fp32-accumulation noise.
  - L2/L3: activations and weights are exact +-1 in fp16; sums of +-1 over
    2048 terms accumulate exactly in fp32 PSUM (integers < 2^24).
  - L4: y3/W4 in fp16 (2^-12 relative), log-softmax in fp32.
"""

import sys

sys.path.insert(0, "/opt/trn_rl_repo")

import numpy as np

B, D, H, C = 8192, 3 * 32 * 32, 2048, 10
EPS = 1e-5
NCORES = 8
BS = B // NCORES          # batch rows per core
KD = D // 128             # 24 k-tiles over input dim
KH = H // 128             # 16 k-tiles over hidden dim
NB = BS // 512            # 2 free-dim chunks of 512

_CACHE = {}
USE_3D_WDMA = True
USE_DR = False  # fp8 DoubleRow for L2/L3 (off: fp16 everywhere)
M_LIMIT = KH  # debug knob: number of m-tiles per layer


def _build(stage=7, fast=(False, False)):
    import concourse.bacc as bacc
    import concourse.mybir as mybir
    import concourse.tile as tile

    F32 = mybir.dt.float32
    F16 = mybir.dt.float16
    F8 = mybir.dt.float8e4
    DR = mybir.MatmulPerfMode.DoubleRow
    ACT = mybir.ActivationFunctionType
    ALU = mybir.AluOpType
    RG = [list(range(NCORES))]

    nc = bacc.Bacc("TRN2", target_bir_lowering=False, debug=False, num_devices=NCORES)

    # ---- I/O ----
    xhi_d = nc.dram_tensor("xT_hi", [D, BS], F16, kind="ExternalInput").ap()
    xlo_d = nc.dram_tensor("xT_lo", [D, BS], F16, kind="ExternalInput").ap()
    w1t_d = nc.dram_tensor("W1T", [D, H], F32, kind="ExternalInput").ap()
    w2t_d = nc.dram_tensor("W2T", [H, H], F32, kind="ExternalInput").ap()
    w3t_d = nc.dram_tensor("W3T", [H, H], F32, kind="ExternalInput").ap()
    CNAMES = ("b1", "g1", "bt1", "b2", "g2", "bt2", "b3", "g3", "bt3")
    # all per-feature BN/bias vectors packed host-side into one tensor
    cpk_d = nc.dram_tensor("cpk", [128, KH * len(CNAMES)], F32, kind="ExternalInput").ap()
    w4pk_d = nc.dram_tensor("w4pk", [128, C * KH], F32, kind="ExternalInput").ap()
    b4_d = nc.dram_tensor("c_b4", [16, 1], F32, kind="ExternalInput").ap()
    out_d = nc.dram_tensor("outT", [C, BS], F32, kind="ExternalOutput").ap()

    wl_d = {1: w1t_d, 2: w2t_d, 3: w3t_d}
    kl = {1: KD, 2: KH, 3: KH}          # contraction k-tiles per layer
    # DRAM scratch holding pre-signed fp8 weights for the DoubleRow layers
    ws8_d = {
        2: nc.dram_tensor("w2s8", [H, H], F8).ap(),
        3: nc.dram_tensor("w3s8", [H, H], F8).ap(),
    }

    with tile.TileContext(nc) as tc:
        with (
            tc.tile_pool(name="pconst", bufs=1) as pconst,
            tc.tile_pool(name="pstat", bufs=1) as pstat,
            tc.tile_pool(name="plog", bufs=1) as plog,
            tc.tile_pool(name="pscr", bufs=3) as pscr,
            tc.tile_pool(name="pw32", bufs=2) as pw32,
            tc.tile_pool(name="pw16", bufs=2) as pw16,
            tc.tile_pool(name="ph", bufs=1) as ph,
            tc.tile_pool(name="pb", bufs=1) as pb,
            tc.tile_pool(name="pa", bufs=1) as pa,
            tc.tile_pool(name="ppsum", bufs=8, space="PSUM") as ppsum,
            tc.tile_pool(name="pdram", bufs=6, space="DRAM") as pdram,
        ):
            # ---- load x.T pieces first: one big 3D-AP DMA per piece ----
            xhi = pa.tile([128, KD * BS], F16, tag="pa")
            xlo = pb.tile([128, KD * BS], F16, tag="pb")
            nc.sync.dma_start(
                xhi[:].rearrange("p (k c) -> p k c", c=BS),
                xhi_d.rearrange("(k p) c -> p k c", p=128),
            )
            nc.sync.dma_start(
                xlo[:].rearrange("p (k c) -> p k c", c=BS),
                xlo_d.rearrange("(k p) c -> p k c", p=128),
            )

            # ---- constants (single packed DMA) ----
            cpk = pconst.tile([128, KH * len(CNAMES)], F32, tag="cpk")
            nc.sync.dma_start(cpk[:], cpk_d)
            cons = {name: cpk[:, i * KH:(i + 1) * KH] for i, name in enumerate(CNAMES)}
            b4s = pconst.tile([16, 1], F32, tag="b4")
            nc.sync.dma_start(b4s[:], b4_d)
            ones10 = pconst.tile([16, 1], F32, tag="ones10")
            nc.vector.memset(ones10[:], 1.0)
            w4st = pconst.tile([128, C * KH], F32, tag="w4st")
            nc.sync.dma_start(w4st[:], w4pk_d)
            w4f = pconst.tile([128, C * KH], F16, tag="w4f")
            nc.vector.tensor_copy(w4f[:], w4st[:])

            parts = {}
            glob = {}

            def dense_layer(l, rhs_hi, rhs_lo):
                """h_l.T = sign(W_l).T-block matmuls; returns SBUF h tile + parts.

                l == 1: fp16 hi/lo 2D path.  l >= 2: fp8 DoubleRow 3D path
                (rhs_hi is a 3D [128, K, BS] fp8 tile of +-1 values).
                """
                K = kl[l]
                dr = USE_DR and l >= 2
                wt_d = wl_d[l]
                h_t = ph.tile([128, KH * BS], F32, tag="ph", name=f"h{l}")
                parts_l = pstat.tile([128, 64], F32, tag=f"parts{l}", name=f"parts{l}")
                if l < 3 and fast[l - 1]:
                    nc.vector.memset(parts_l[:, 32:64], 0.0)
                bias_t = cons[f"b{l}"]
                n_kg = K // 8  # kgroups of 8 k-tiles
                for m in range(M_LIMIT):
                    if dr:
                        # fp8 sign weights were pre-signed to DRAM during L1
                        w16 = pw16.tile([128, K * 128], F8, tag="w16", name=f"w8_{l}_{m}")
                        nc.sync.dma_start(w16[:], ws8_d[l][m * 128:(m + 1) * 128, :])
                        w8v = w16[:].rearrange("p (k c) -> p k c", c=128)
                    else:
                        w16 = pw16.tile([128, K * 128], F16, tag="w16", name=f"w16_{l}_{m}")
                        for kg in range(n_kg):
                            wst = pw32.tile([128, 1024], F32, tag="w32", name=f"wst_{l}_{m}_{kg}")
                            if USE_3D_WDMA:
                                src = wt_d[kg * 1024:(kg + 1) * 1024, m * 128:(m + 1) * 128]
                                nc.sync.dma_start(
                                    wst[:].rearrange("p (j c) -> p j c", j=8),
                                    src.rearrange("(j p) c -> p j c", p=128),
                                )
                            else:
                                for j in range(8):
                                    k = kg * 8 + j
                                    nc.sync.dma_start(
                                        wst[:, j * 128:(j + 1) * 128],
                                        wt_d[k * 128:(k + 1) * 128, m * 128:(m + 1) * 128],
                                    )
                            nc.scalar.activation(w16[:, kg * 1024:(kg + 1) * 1024], wst[:], ACT.Sign)
                    for n in range(NB):
                        ps = ppsum.tile([128, 512], F32, tag="ps", name=f"ps_{l}_{m}_{n}")
                        if dr:
                            for t in range(K // 2):
                                nc.tensor.matmul(
                                    ps[:], w8v[:, 2 * t:2 * t + 2, :],
                                    rhs_hi[:, 2 * t:2 * t + 2, n * 512:n * 512 + 512],
                                    start=(t == 0), stop=(t == K // 2 - 1), perf_mode=DR)
                        else:
                            # per k: one weight load feeds both hi and lo matmuls
                            for k in range(K):
                                lhsT = w16[:, k * 128:(k + 1) * 128]
                                sl = slice(k * BS + n * 512, k * BS + n * 512 + 512)
                                nc.tensor.matmul(ps[:], lhsT, rhs_hi[:, sl], start=(k == 0), stop=(rhs_lo is None and k == K - 1))
                                if rhs_lo is not None:
                                    nc.tensor.matmul(ps[:], lhsT, rhs_lo[:, sl], start=False, stop=(k == K - 1))
                        hs = h_t[:, m * BS + n * 512: m * BS + n * 512 + 512]
                        col = 2 * m + n
                        nc.scalar.activation(hs, ps[:], ACT.Identity, bias=bias_t[:, m:m + 1],
                                             scale=1.0, accum_out=parts_l[:, col:col + 1])
                        if not (l < 3 and fast[l - 1]):
                            scr = pscr.tile([128, BS], F32, tag="scr", name=f"sq_{l}_{m}_{n}")
                            nc.scalar.activation(scr[:, :512], hs, ACT.Square,
                                                 accum_out=parts_l[:, 32 + col:32 + col + 1])
                parts[l] = parts_l
                return h_t

            def bn_stats(l):
                """AllReduce parts -> per-feature scale rp (=g*rsqrt(v+eps)) and bias c."""
                arin = pdram.tile([128, 64], F32, tag=f"arin{l}")
                arout = pdram.tile([128, 64], F32, tag=f"arout{l}")
                nc.sync.dma_start(arin[:], parts[l][:])
                nc.gpsimd.collective_compute(
                    "AllReduce", ALU.add, replica_groups=RG,
                    ins=[arin.opt()], outs=[arout.opt()])
                g_t = pstat.tile([128, 64], F32, tag=f"glob{l}")
                nc.sync.dma_start(g_t[:], arout[:])
                glob[l] = g_t

                def st(tag):
                    return pstat.tile([128, KH], F32, name=f"{tag}{l}", tag=f"{tag}{l}")

                if l < 3 and fast[l - 1]:
                    # bt==0 and g>0: sign threshold is just the batch mean
                    sg, m1, negm = st("sg"), st("m1"), st("negm")
                    nc.vector.tensor_reduce(sg[:], g_t[:, 0:32].rearrange("p (m n) -> p m n", n=2),
                                            axis=mybir.AxisListType.X, op=ALU.add)
                    nc.vector.tensor_scalar_mul(m1[:], sg[:], 1.0 / B)
                    nc.vector.tensor_scalar_mul(negm[:], sg[:], -1.0 / B)
                    return None, negm, m1, None, None

                sg, qg, m1, msq, m1sq, v, sq, r, rp, mt, c, u, u2, tthr, s, s2, sneg = (
                    st(x) for x in ("sg", "qg", "m1", "msq", "m1sq", "v", "sq", "r",
                                    "rp", "mt", "c", "u", "u2", "tthr", "s", "s2", "sneg"))
                nc.vector.tensor_reduce(sg[:], g_t[:, 0:32].rearrange("p (m n) -> p m n", n=2),
                                        axis=mybir.AxisListType.X, op=ALU.add)
                nc.vector.tensor_reduce(qg[:], g_t[:, 32:64].rearrange("p (m n) -> p m n", n=2),
                                        axis=mybir.AxisListType.X, op=ALU.add)
                nc.vector.tensor_scalar_mul(m1[:], sg[:], 1.0 / B)
                nc.vector.tensor_scalar_mul(msq[:], qg[:], 1.0 / B)
                nc.vector.tensor_tensor(m1sq[:], m1[:], m1[:], op=ALU.mult)
                nc.vector.tensor_tensor(v[:], msq[:], m1sq[:], op=ALU.subtract)
                nc.vector.tensor_scalar_add(v[:], v[:], EPS)
                nc.scalar.activation(sq[:], v[:], ACT.Sqrt)
                nc.vector.reciprocal(r[:], sq[:])
                nc.vector.tensor_tensor(rp[:], cons[f"g{l}"][:], r[:], op=ALU.mult)
                nc.vector.tensor_tensor(mt[:], m1[:], rp[:], op=ALU.mult)
                nc.vector.tensor_tensor(c[:], cons[f"bt{l}"][:], mt[:], op=ALU.subtract)
                # DVE-path sign params: a = is_ge(h, t)*2s - s with t = m - bt/(g*r)
                gi = st("gi")
                nc.vector.reciprocal(gi[:], cons[f"g{l}"][:])
                nc.vector.tensor_tensor(u[:], cons[f"bt{l}"][:], gi[:], op=ALU.mult)
                nc.vector.tensor_tensor(u2[:], u[:], sq[:], op=ALU.mult)
                nc.vector.tensor_tensor(tthr[:], m1[:], u2[:], op=ALU.subtract)
                nc.scalar.activation(s[:], cons[f"g{l}"][:], ACT.Sign)
                nc.vector.tensor_scalar_mul(s2[:], s[:], 2.0)
                nc.vector.tensor_scalar_mul(sneg[:], s[:], -1.0)
                return rp, c, tthr, s2, sneg

            def debug_out(src_ap, cast=False):
                """DMA a [C, BS] f32 view to out for stage bisection."""
                if cast:
                    t = pscr.tile([128, BS], F32, tag="scr", name="dbgcast")
                    nc.vector.tensor_copy(t[:C, :], src_ap)
                    src_ap = t[:C, :]
                nc.sync.dma_start(out_d[:], src_ap)


            def sign_wave(dst_tile, h_t, rp, c, tthr, s2, sneg, dr_mode, tagp):
                fastp = rp is None   # c = -m (ACT bias), tthr = m (DVE threshold)
                for k in range(KH):
                    hsl = h_t[:, k * BS:(k + 1) * BS]
                    dst = dst_tile[:, k, :] if dr_mode else dst_tile[:, k * BS:(k + 1) * BS]
                    if k < 10:
                        scale = 1.0 if fastp else rp[:, k:k + 1]
                        nc.scalar.activation(dst, hsl, ACT.Sign, bias=c[:, k:k + 1], scale=scale)
                    else:
                        b = pscr.tile([128, BS], F16, tag="scr", name=f"sgb_{tagp}_{k}")
                        nc.vector.tensor_scalar(out=b[:], in0=hsl, scalar1=tthr[:, k:k + 1],
                                                scalar2=None, op0=ALU.is_ge)
                        s2a = 2.0 if fastp else s2[:, k:k + 1]
                        sna = -1.0 if fastp else sneg[:, k:k + 1]
                        nc.vector.tensor_scalar(out=dst, in0=b[:], scalar1=s2a,
                                                scalar2=sna, op0=ALU.mult, op1=ALU.add)

            # ===== Layer 1 =====
            h1 = dense_layer(1, xhi, xlo)

            # Background sign pre-pass: W2/W3 fp32 -> fp8 signs in DRAM.
            # Emitted after L1 so it runs at lower priority in L1's DMA/ACT gaps.
            if USE_DR and stage >= 3:
                for l in (2, 3):
                    for m in range(M_LIMIT):
                        for kg in range(2):
                            wst = pw32.tile([128, 1024], F32, tag="w32", name=f"pre32_{l}_{m}_{kg}")
                            src = wl_d[l][kg * 1024:(kg + 1) * 1024, m * 128:(m + 1) * 128]
                            nc.sync.dma_start(
                                wst[:].rearrange("p (j c) -> p j c", j=8),
                                src.rearrange("(j p) c -> p j c", p=128),
                            )
                            s8 = pscr.tile([128, 1024], F8, tag="scr", name=f"pre8_{l}_{m}_{kg}")
                            nc.scalar.activation(s8[:], wst[:], ACT.Sign)
                            nc.sync.dma_start(
                                ws8_d[l][m * 128:(m + 1) * 128, kg * 1024:(kg + 1) * 1024],
                                s8[:],
                            )

            if stage == 1:
                debug_out(h1[:C, :BS])
            if stage >= 2:
                rp1, c1, t1, s21, sn1 = bn_stats(1)
                if USE_DR:
                    a2 = pa.tile([128, KH, BS], F8, tag="pa", name="a2")   # reuses xT_hi slot
                else:
                    a2 = pa.tile([128, KH * BS], F16, tag="pa", name="a2")
                sign_wave(a2, h1, rp1, c1, t1, s21, sn1, USE_DR, "a2")
                if stage == 2:
                    debug_out(a2[:C, 0, :] if USE_DR else a2[:C, :BS], cast=True)

            if stage >= 3:
                # ===== Layer 2 =====
                h2 = dense_layer(2, a2, None)
                rp2, c2, t2, s22, sn2 = bn_stats(2)
                if USE_DR:
                    a3 = pb.tile([128, KH, BS], F8, tag="pb", name="a3")   # reuses xT_lo slot
                else:
                    a3 = pb.tile([128, KH * BS], F16, tag="pb", name="a3")
                sign_wave(a3, h2, rp2, c2, t2, s22, sn2, USE_DR, "a3")
                if stage == 3:
                    debug_out(a3[:C, 0, :] if USE_DR else a3[:C, :BS], cast=True)

            if stage >= 4:
                # ===== Layer 3 =====
                h3 = dense_layer(3, a3, None)
                rp3, c3, _t3, _s23, _sn3 = bn_stats(3)
                y3 = pa.tile([128, KH * BS], F16, tag="pa")   # reuses a2 slot
                for k in range(KH):
                    scr = pscr.tile([128, BS], F32, tag="scr")
                    nc.scalar.activation(scr[:], h3[:, k * BS:(k + 1) * BS],
                                         ACT.Identity, bias=c3[:, k:k + 1], scale=rp3[:, k:k + 1])
                    nc.vector.tensor_scalar(out=y3[:, k * BS:(k + 1) * BS], in0=scr[:],
                                            scalar1=-1.0, scalar2=1.0, op0=ALU.max, op1=ALU.min)
                if stage == 4:
                    debug_out(y3[:C, :BS], cast=True)

            if stage >= 5:
                # ===== Layer 4 + log-softmax =====
                logits = plog.tile([16, BS], F32, tag="logits")
                for n in range(NB):
                    ps4 = ppsum.tile([128, 512], F32, tag="ps")
                    for k in range(KH):
                        nc.tensor.matmul(ps4[:C, :], w4f[:, k * C:(k + 1) * C],
                                         y3[:, k * BS + n * 512: k * BS + n * 512 + 512],
                                         start=(k == 0), stop=(k == KH - 1))
                    nc.scalar.activation(logits[:C, n * 512:(n + 1) * 512], ps4[:C, :],
                                         ACT.Identity, bias=b4s[:C, :], scale=1.0)
                if stage == 5:
                    debug_out(logits[:C, :])

            if stage >= 6:
                e_t = pscr.tile([128, BS], F32, tag="scr")
                nc.scalar.activation(e_t[:C, :], logits[:C, :], ACT.Exp)
                lse = pscr.tile([128, BS], F32, tag="scr")
                for n in range(NB):
                    ps5 = ppsum.tile([128, 512], F32, tag="ps")
                    nc.tensor.matmul(ps5[:1, :], ones10[:C, :], e_t[:C, n * 512:(n + 1) * 512],
                                     start=True, stop=True)
                    nc.scalar.activation(lse[:1, n * 512:(n + 1) * 512], ps5[:1, :], ACT.Ln)
                lse10 = pscr.tile([128, BS], F32, tag="scr")
                nc.gpsimd.partition_broadcast(lse10[:C, :], lse[:1, :], channels=C)
                outs = plog.tile([16, BS], F32, tag="outs")
                nc.vector.tensor_tensor(outs[:C, :], logits[:C, :], lse10[:C, :], op=ALU.subtract)
                nc.sync.dma_start(out_d[:], outs[:C, :])

    nc.compile()
    return nc


def _prep_inputs(x, W1, b1, g1, bt1, W2, b2, g2, bt2, W3, b3, g3, bt3, W4, b4):
    """Host-side sharding + layout prep (pure layout/permutation + lossless split)."""
    def as32(a):
        return np.ascontiguousarray(np.asarray(a, dtype=np.float32))

    x = as32(x)
    shared = {
        "W1T": np.ascontiguousarray(as32(W1).T),
        "W2T": np.ascontiguousarray(as32(W2).T),
        "W3T": np.ascontiguousarray(as32(W3).T),
    }
    cvecs = (b1, g1, bt1, b2, g2, bt2, b3, g3, bt3)
    cpk = np.empty((128, KH * len(cvecs)), np.float32)
    for i, v in enumerate(cvecs):
        cpk[:, i * KH:(i + 1) * KH] = as32(v).reshape(KH, 128).T
    shared["cpk"] = cpk
    w4T = np.ascontiguousarray(as32(W4).T)          # [H, C]
    w4pk = np.empty((128, C * KH), np.float32)
    for k in range(KH):
        w4pk[:, k * C:(k + 1) * C] = w4T[k * 128:(k + 1) * 128, :]
    shared["w4pk"] = w4pk
    b4p = np.zeros((16, 1), np.float32)
    b4p[:C, 0] = as32(b4).reshape(-1)
    shared["c_b4"] = b4p

    in_maps = []
    for c in range(NCORES):
        xT = np.ascontiguousarray(x[c * BS:(c + 1) * BS].T)     # [D, BS]
        hi = xT.astype(np.float16)
        lo = (xT - hi.astype(np.float32)).astype(np.float16)    # exact residual fits fp16
        m = dict(shared)
        m["xT_hi"] = hi
        m["xT_lo"] = lo
        in_maps.append(m)
    return in_maps


def _fast_flags(inputs):
    """Mean-only BN boundaries are valid when beta==0 and gamma>0 (sign(g*r*(h-m)) == sign(h-m))."""
    def ok(g, bt):
        g, bt = np.asarray(g), np.asarray(bt)
        return bool(not np.any(bt) and np.all(g > 0))

    return (ok(inputs["g1"], inputs["bt1"]), ok(inputs["g2"], inputs["bt2"]))


def kernel(**inputs) -> np.ndarray:
    from concourse.bass_utils import run_bass_kernel_spmd

    fast = _fast_flags(inputs)
    if _CACHE.get("fast") != fast:
        _CACHE["nc"] = _build(fast=fast)
        _CACHE["fast"] = fast
    nc = _CACHE["nc"]
    in_maps = _prep_inputs(**inputs)
    res = run_bass_kernel_spmd(nc, in_maps, list(range(NCORES)))
    out = np.concatenate([res.results[c]["outT"].T for c in range(NCORES)], axis=0)
    return out.astype(np.float32)



# revision 2
# speedup vs baseline: 1.4215x; 1.4215x over previous
"""Trainium2 Bass kernel for nn_BinarizedCifar10MLP.

Data-parallel over batch (8192/8 = 1024 rows/core), feature-major activation
layout [features, batch] on device.  BN batch statistics are all-reduced in two
chunks per layer so the collective latency hides under the layer's tail
matmuls; the next layer's matmuls are emitted k-phased (k < SPLIT*128 first for
the first 8 PSUM groups) so they start before the late stats chunk lands.

Precision scheme (reference is fp32, gate rel_err < 2e-2):
  - Weights are sign(+-1), pre-signed on the host: W1 as fp16 (exact), W2/W3 as
    fp8e4m3 (exact +-1) driven in DoubleRow mode at 2x PE rate.
  - L1 (x @ sign(W1).T): x split on host into fp16 hi + lo pieces.  lo either
    exact fp16 (L1MODE=hi16lo16) or fp8e5m2 of lo*2^11 matched with
    sign(W1)*2^-11 fp8e5m2 weights in DoubleRow mode (L1MODE=hi16lo8), which
    accumulates into the same fp32 PSUM group as hi.
  - L2/L3: +-1 x +-1 products accumulate exactly in fp32 PSUM.
  - L4: y3/W4 in fp16, log-softmax in fp32.
"""

import sys

sys.path.insert(0, "/opt/trn_rl_repo")

import numpy as np
import ml_dtypes

B, D, H, C = 8192, 3 * 32 * 32, 2048, 10
EPS = 1e-5
NCORES = 8
BS = B // NCORES          # batch rows per core
KD = D // 128             # 24 k-tiles over input dim
KH = H // 128             # 16 k-tiles over hidden dim
NB = BS // 512            # 2 free-dim chunks of 512
SPLIT = 12                # m-tiles covered by the early stats AllReduce chunk
P1G = 8                   # psum groups in the k-phased prologue of L2/L3
XCH = 4                   # x DMA chunks
ACT_EVERY = 4             # every ACT_EVERY-th sign/y3 tile goes to ScalarE

L1MODE = "hi16lo8"        # "hi16lo16" (exact) | "hi16lo8" (fp8 DoubleRow lo)

_CACHE = {}


def _build(stage=7, fast=(False, False), l1mode=None):
    import concourse.bacc as bacc
    import concourse.mybir as mybir
    import concourse.tile as tile

    l1mode = l1mode or L1MODE
    lo8 = l1mode == "hi16lo8"
    F32 = mybir.dt.float32
    F16 = mybir.dt.float16
    F8E4 = mybir.dt.float8e4
    F8E5 = mybir.dt.float8e5
    DR = mybir.MatmulPerfMode.DoubleRow
    ACT = mybir.ActivationFunctionType
    ALU = mybir.AluOpType
    RG = [list(range(NCORES))]

    nc = bacc.Bacc("TRN2", target_bir_lowering=False, debug=False, num_devices=NCORES)

    # ---- I/O ----
    xhi_d = nc.dram_tensor("xT_hi", [D, BS], F16, kind="ExternalInput").ap()
    if lo8:
        xlo_d = nc.dram_tensor("xT_lo8", [D, BS], F8E5, kind="ExternalInput").ap()
        w1lo_d = nc.dram_tensor("w1lopk", [128, KH * KD * 128], F8E5, kind="ExternalInput").ap()
    else:
        xlo_d = nc.dram_tensor("xT_lo", [D, BS], F16, kind="ExternalInput").ap()
        w1lo_d = None
    w1pk_d = nc.dram_tensor("w1pk", [128, KH * KD * 128], F16, kind="ExternalInput").ap()
    w2pk_d = nc.dram_tensor("w2pk", [128, KH * KH * 128], F8E4, kind="ExternalInput").ap()
    w3pk_d = nc.dram_tensor("w3pk", [128, KH * KH * 128], F8E4, kind="ExternalInput").ap()
    CNAMES = ("b1", "g1", "bt1", "b2", "g2", "bt2", "b3", "g3", "bt3")
    cpk_d = nc.dram_tensor("cpk", [128, KH * len(CNAMES)], F32, kind="ExternalInput").ap()
    w4pk_d = nc.dram_tensor("w4pk", [128, C * KH], F16, kind="ExternalInput").ap()
    b4_d = nc.dram_tensor("c_b4", [16, 1], F32, kind="ExternalInput").ap()
    out_d = nc.dram_tensor("outT", [C, BS], F32, kind="ExternalOutput").ap()
    wpk_d = {2: w2pk_d, 3: w3pk_d}

    with tile.TileContext(nc) as tc:
        with (
            tc.tile_pool(name="pconst", bufs=1) as pconst,
            tc.tile_pool(name="pstat", bufs=1) as pstat,
            tc.tile_pool(name="plog", bufs=1) as plog,
            tc.tile_pool(name="pscr", bufs=2) as pscr,
            tc.tile_pool(name="pw1", bufs=2) as pw1,
            tc.tile_pool(name="pw1lo", bufs=2) as pw1lo,
            tc.tile_pool(name="pw8", bufs=5) as pw8,
            tc.tile_pool(name="pa", bufs=1) as pa,
            tc.tile_pool(name="pb", bufs=1) as pb,
            tc.tile_pool(name="pa2", bufs=1) as pa2,
            tc.tile_pool(name="ph", bufs=1) as ph,
            tc.tile_pool(name="ppsum", bufs=8, space="PSUM") as ppsum,
            tc.tile_pool(name="pdram", bufs=1, space="DRAM") as pdram,
        ):
            # ---- load x pieces in chunks so the first matmuls start early ----
            xhi = pa.tile([128, KD * BS], F16, tag="pa", name="xhi")
            if lo8:
                xlo = pb.tile([128, KD * BS], F8E5, tag="pb", name="xlo")
            else:
                xlo = pb.tile([128, KD * BS], F16, tag="pb", name="xlo")
            xhiv = xhi[:].rearrange("p (k c) -> p k c", c=BS)
            xlov = xlo[:].rearrange("p (k c) -> p k c", c=BS)
            xhisrc = xhi_d.rearrange("(k p) c -> p k c", p=128)
            xlosrc = xlo_d.rearrange("(k p) c -> p k c", p=128)
            kpc = KD // XCH
            for ch in range(XCH):
                k0, k1 = ch * kpc, (ch + 1) * kpc
                nc.sync.dma_start(xhiv[:, k0:k1, :], xhisrc[:, k0:k1, :])
            for ch in range(XCH):
                k0, k1 = ch * kpc, (ch + 1) * kpc
                nc.sync.dma_start(xlov[:, k0:k1, :], xlosrc[:, k0:k1, :])

            # ---- constants ----
            cpk = pconst.tile([128, KH * len(CNAMES)], F32, tag="cpk")
            nc.sync.dma_start(cpk[:], cpk_d)
            cons = {name: cpk[:, i * KH:(i + 1) * KH] for i, name in enumerate(CNAMES)}
            b4s = pconst.tile([16, 1], F32, tag="b4")
            nc.sync.dma_start(b4s[:], b4_d)
            ones10 = pconst.tile([16, 1], F32, tag="ones10")
            nc.vector.memset(ones10[:], 1.0)
            w4f = pconst.tile([128, C * KH], F16, tag="w4f")
            nc.sync.dma_start(w4f[:], w4pk_d)

            parts = {}
            stats = {}

            def st(l, tag):
                key = (l, tag)
                if key not in stats:
                    stats[key] = pstat.tile([128, KH], F32, name=f"{tag}{l}", tag=f"{tag}{l}")
                return stats[key]

            def is_fast(l):
                return l < 3 and fast[l - 1]

            def stats_chunk(l, m0, m1, g_t):
                """g_t: [128, 2d] (fast) or [128, 4d] (full): [sums | sqsums]."""
                d = m1 - m0
                red = pstat.tile([128, d], F32, tag=f"red{l}{m0}", name=f"red{l}{m0}")
                nc.vector.tensor_reduce(
                    red[:], g_t[:, 0:2 * d].rearrange("p (m n) -> p m n", n=2),
                    axis=mybir.AxisListType.X, op=ALU.add)
                if is_fast(l):
                    thr, negm = st(l, "thr"), st(l, "negm")
                    nc.vector.tensor_scalar_mul(thr[:, m0:m1], red[:], 1.0 / B)
                    nc.vector.tensor_scalar_mul(negm[:, m0:m1], red[:], -1.0 / B)
                    return
                redq = pstat.tile([128, d], F32, tag=f"redq{l}{m0}", name=f"redq{l}{m0}")
                nc.vector.tensor_reduce(
                    redq[:], g_t[:, 2 * d:4 * d].rearrange("p (m n) -> p m n", n=2),
                    axis=mybir.AxisListType.X, op=ALU.add)
                sl = slice(m0, m1)
                m1c, msq, m1sq, v, sq, r, rp, mt, c = (
                    st(l, x) for x in ("m1", "msq", "m1sq", "v", "sq", "r", "rp", "mt", "c"))
                nc.vector.tensor_scalar_mul(m1c[:, sl], red[:], 1.0 / B)
                nc.vector.tensor_scalar_mul(msq[:, sl], redq[:], 1.0 / B)
                nc.vector.tensor_tensor(m1sq[:, sl], m1c[:, sl], m1c[:, sl], op=ALU.mult)
                nc.vector.tensor_tensor(v[:, sl], msq[:, sl], m1sq[:, sl], op=ALU.subtract)
                nc.vector.tensor_scalar_add(v[:, sl], v[:, sl], EPS)
                nc.scalar.activation(sq[:, sl], v[:, sl], ACT.Sqrt)
                nc.vector.reciprocal(r[:, sl], sq[:, sl])
                nc.vector.tensor_tensor(rp[:, sl], cons[f"g{l}"][:, sl], r[:, sl], op=ALU.mult)
                nc.vector.tensor_tensor(mt[:, sl], m1c[:, sl], rp[:, sl], op=ALU.mult)
                nc.vector.tensor_tensor(c[:, sl], cons[f"bt{l}"][:, sl], mt[:, sl], op=ALU.subtract)
                if l < 3:
                    # DVE sign params: threshold t = m - bt/(g*r), a = is_ge*2s - s
                    gi, u, u2, tthr, s, s2, sneg = (
                        st(l, x) for x in ("gi", "u", "u2", "tthr", "s", "s2", "sneg"))
                    nc.vector.reciprocal(gi[:, sl], cons[f"g{l}"][:, sl])
                    nc.vector.tensor_tensor(u[:, sl], cons[f"bt{l}"][:, sl], gi[:, sl], op=ALU.mult)
                    nc.vector.tensor_tensor(u2[:, sl], u[:, sl], sq[:, sl], op=ALU.mult)
                    nc.vector.tensor_tensor(tthr[:, sl], m1c[:, sl], u2[:, sl], op=ALU.subtract)
                    nc.scalar.activation(s[:, sl], cons[f"g{l}"][:, sl], ACT.Sign)
                    nc.vector.tensor_scalar_mul(s2[:, sl], s[:, sl], 2.0)
                    nc.vector.tensor_scalar_mul(sneg[:, sl], s[:, sl], -1.0)

            def boundary(l, m0, m1, tag):
                """AllReduce parts cols for m-tiles [m0, m1) and compute stats."""
                d = m1 - m0
                w = 2 * d if is_fast(l) else 4 * d
                arin = pdram.tile([128, w], F32, tag=f"arin{l}{tag}")
                arout = pdram.tile([128, w], F32, tag=f"arout{l}{tag}")
                nc.sync.dma_start(arin[:, 0:2 * d], parts[l][:, 2 * m0:2 * m1])
                if not is_fast(l):
                    nc.sync.dma_start(arin[:, 2 * d:4 * d], parts[l][:, 32 + 2 * m0:32 + 2 * m1])
                nc.gpsimd.collective_compute(
                    "AllReduce", ALU.add, replica_groups=RG,
                    ins=[arin.opt()], outs=[arout.opt()])
                g_t = pstat.tile([128, w], F32, tag=f"g{l}{tag}", name=f"g{l}{tag}")
                nc.sync.dma_start(g_t[:], arout[:])
                stats_chunk(l, m0, m1, g_t)

            def sign_chunk(l, dst3, h_t, k0, k1, use_act):
                fastl = is_fast(l)
                for k in range(k0, k1):
                    hsl = h_t[:, k * BS:(k + 1) * BS]
                    dst = dst3[:, k, :]
                    if use_act and (k % ACT_EVERY == ACT_EVERY - 1):
                        scale = 1.0 if fastl else st(l, "rp")[:, k:k + 1]
                        bias = st(l, "negm" if fastl else "c")[:, k:k + 1]
                        nc.scalar.activation(dst, hsl, ACT.Sign, bias=bias, scale=scale)
                    else:
                        thr = st(l, "thr" if fastl else "tthr")[:, k:k + 1]
                        b = pscr.tile([128, BS], F16, tag="scr", name=f"sgb_{l}_{k}")
                        nc.vector.tensor_scalar(out=b[:], in0=hsl, scalar1=thr,
                                                scalar2=None, op0=ALU.is_ge)
                        s2a = 2.0 if fastl else st(l, "s2")[:, k:k + 1]
                        sna = -1.0 if fastl else st(l, "sneg")[:, k:k + 1]
                        nc.vector.tensor_scalar(out=dst, in0=b[:], scalar1=s2a,
                                                scalar2=sna, op0=ALU.mult, op1=ALU.add)

            def y3_chunk(dst, h_t, k0, k1, use_act):
                rp3, c3 = st(3, "rp"), st(3, "c")
                for k in range(k0, k1):
                    hsl = h_t[:, k * BS:(k + 1) * BS]
                    scr = pscr.tile([128, BS], F32, tag="scr", name=f"y3s_{k}")
                    if use_act and (k % ACT_EVERY == ACT_EVERY - 1):
                        nc.scalar.activation(scr[:], hsl, ACT.Identity,
                                             bias=c3[:, k:k + 1], scale=rp3[:, k:k + 1])
                    else:
                        nc.vector.tensor_scalar(out=scr[:], in0=hsl, scalar1=rp3[:, k:k + 1],
                                                scalar2=c3[:, k:k + 1], op0=ALU.mult, op1=ALU.add)
                    nc.vector.tensor_scalar(out=dst[:, k * BS:(k + 1) * BS], in0=scr[:],
                                            scalar1=-1.0, scalar2=1.0, op0=ALU.max, op1=ALU.min)

            def finish_group(l, h_t, ps, m, n):
                hs = h_t[:, m * BS + n * 512: m * BS + n * 512 + 512]
                col = 2 * m + n
                nc.scalar.activation(hs, ps[:], ACT.Identity, bias=cons[f"b{l}"][:, m:m + 1],
                                     scale=1.0, accum_out=parts[l][:, col:col + 1])
                if not is_fast(l):
                    scr = pscr.tile([128, BS], F32, tag="scr", name=f"sq_{l}_{m}_{n}")
                    nc.scalar.activation(scr[:, :512], hs, ACT.Square,
                                         accum_out=parts[l][:, 32 + col:32 + col + 1])

            # ===================== Layer 1 =====================
            h1 = ph.tile([128, KH * BS], F32, tag="ph", name="h1")
            parts[1] = pstat.tile([128, 64], F32, tag="parts1", name="parts1")
            a2 = pa2.tile([128, KH, BS], F8E4, tag="pa2", name="a2")
            for m in range(KH):
                w16 = pw1.tile([128, KD * 128], F16, tag="w1", name=f"w16_{m}")
                nc.sync.dma_start(w16[:], w1pk_d[:, m * KD * 128:(m + 1) * KD * 128])
                if lo8:
                    wlo = pw1lo.tile([128, KD * 128], F8E5, tag="w1lo", name=f"wlo_{m}")
                    nc.sync.dma_start(wlo[:], w1lo_d[:, m * KD * 128:(m + 1) * KD * 128])
                    wlov = wlo[:].rearrange("p (k c) -> p k c", c=128)
                for n in range(NB):
                    ps = ppsum.tile([128, 512], F32, tag="ps", name=f"ps_1_{m}_{n}")
                    if lo8:
                        for k in range(KD):
                            nc.tensor.matmul(ps[:], w16[:, k * 128:(k + 1) * 128],
                                             xhi[:, k * BS + n * 512: k * BS + n * 512 + 512],
                                             start=(k == 0), stop=False)
                        for t in range(KD // 2):
                            nc.tensor.matmul(ps[:], wlov[:, 2 * t:2 * t + 2, :],
                                             xlov[:, 2 * t:2 * t + 2, n * 512:n * 512 + 512],
                                             start=False, stop=(t == KD // 2 - 1), perf_mode=DR)
                    else:
                        for k in range(KD):
                            lhsT = w16[:, k * 128:(k + 1) * 128]
                            sl = slice(k * BS + n * 512, k * BS + n * 512 + 512)
                            nc.tensor.matmul(ps[:], lhsT, xhi[:, sl], start=(k == 0), stop=False)
                            nc.tensor.matmul(ps[:], lhsT, xlo[:, sl], start=False, stop=(k == KD - 1))
                    finish_group(1, h1, ps, m, n)
                if m == SPLIT - 1 and stage >= 2:
                    boundary(1, 0, SPLIT, "A")
                    if lo8:
                        # sign-wave A overlaps the L1 tail (a2 has its own slot)
                        sign_chunk(1, a2, h1, 0, SPLIT, use_act=False)
            if stage == 1:
                nc.sync.dma_start(out_d[:], h1[:C, :BS])
            if stage >= 2:
                boundary(1, SPLIT, KH, "B")
                if not lo8:
                    sign_chunk(1, a2, h1, 0, SPLIT, use_act=True)
                sign_chunk(1, a2, h1, SPLIT, KH, use_act=True)
                if stage == 2:
                    t = pscr.tile([128, BS], F32, tag="scr", name="dbg2")
                    nc.vector.tensor_copy(t[:C, :], a2[:C, 0, :])
                    nc.sync.dma_start(out_d[:], t[:C, :])

            # ================= Layers 2 and 3 (DoubleRow fp8) =================
            def dense_dr(l, rhs3, dst3_or_y3):
                h_t = ph.tile([128, KH * BS], F32, tag="ph", name=f"h{l}")
                parts[l] = pstat.tile([128, 64], F32, tag=f"parts{l}", name=f"parts{l}")
                w_tiles = {}

                def ensure_w(m):
                    if m not in w_tiles:
                        w = pw8.tile([128, KH * 128], F8E4, tag="w8", name=f"w8_{l}_{m}")
                        nc.sync.dma_start(w[:], wpk_d[l][:, m * KH * 128:(m + 1) * KH * 128])
                        w_tiles[m] = w[:].rearrange("p (k c) -> p k c", c=128)
                    return w_tiles[m]

                groups = [(m, n) for m in range(KH) for n in range(NB)]
                psums = {}

                def emit(g, t0, t1):
                    m, n = groups[g]
                    wv = ensure_w(m)
                    if g not in psums:
                        psums[g] = ppsum.tile([128, 512], F32, tag="ps", name=f"ps_{l}_{g}")
                    ps = psums[g]
                    for t in range(t0, t1):
                        nc.tensor.matmul(ps[:], wv[:, 2 * t:2 * t + 2, :],
                                         rhs3[:, 2 * t:2 * t + 2, n * 512:n * 512 + 512],
                                         start=(t == 0), stop=(t == KH // 2 - 1), perf_mode=DR)

                TSPLIT = SPLIT // 2
                for g in range(P1G):
                    emit(g, 0, TSPLIT)
                for g in range(P1G):
                    emit(g, TSPLIT, KH // 2)
                    finish_group(l, h_t, psums[g], *groups[g])
                for g in range(P1G, len(groups)):
                    emit(g, 0, KH // 2)
                    finish_group(l, h_t, psums[g], *groups[g])
                    m, n = groups[g]
                    if m == SPLIT - 1 and n == NB - 1:
                        boundary(l, 0, SPLIT, "A")
                        if l < 3:
                            sign_chunk(l, dst3_or_y3, h_t, 0, SPLIT, use_act=False)
                        else:
                            y3_chunk(dst3_or_y3, h_t, 0, SPLIT, use_act=False)
                boundary(l, SPLIT, KH, "B")
                if l < 3:
                    sign_chunk(l, dst3_or_y3, h_t, SPLIT, KH, use_act=True)
                else:
                    y3_chunk(dst3_or_y3, h_t, SPLIT, KH, use_act=True)
                return h_t

            if stage >= 3:
                a3 = pa.tile([128, KH, BS], F8E4, tag="pa", name="a3")
                dense_dr(2, a2[:], a3)
                if stage == 3:
                    t = pscr.tile([128, BS], F32, tag="scr", name="dbg3")
                    nc.vector.tensor_copy(t[:C, :], a3[:C, 0, :])
                    nc.sync.dma_start(out_d[:], t[:C, :])

            if stage >= 4:
                y3 = pb.tile([128, KH * BS], F16, tag="pb", name="y3")
                dense_dr(3, a3[:], y3)
                if stage == 4:
                    t = pscr.tile([128, BS], F32, tag="scr", name="dbg4")
                    nc.vector.tensor_copy(t[:C, :], y3[:C, :BS])
                    nc.sync.dma_start(out_d[:], t[:C, :])

            if stage >= 5:
                # ===== Layer 4 + log-softmax =====
                logits = plog.tile([16, BS], F32, tag="logits")
                ps4s = {}
                for n in range(NB):
                    ps4s[n] = ppsum.tile([128, 512], F32, tag="ps", name=f"ps4_{n}")
                    for k in range(SPLIT):
                        nc.tensor.matmul(ps4s[n][:C, :], w4f[:, k * C:(k + 1) * C],
                                         y3[:, k * BS + n * 512: k * BS + n * 512 + 512],
                                         start=(k == 0), stop=False)
                for n in range(NB):
                    for k in range(SPLIT, KH):
                        nc.tensor.matmul(ps4s[n][:C, :], w4f[:, k * C:(k + 1) * C],
                                         y3[:, k * BS + n * 512: k * BS + n * 512 + 512],
                                         start=False, stop=(k == KH - 1))
                    nc.scalar.activation(logits[:C, n * 512:(n + 1) * 512], ps4s[n][:C, :],
                                         ACT.Identity, bias=b4s[:C, :], scale=1.0)
                if stage == 5:
                    nc.sync.dma_start(out_d[:], logits[:C, :])

            if stage >= 6:
                e_t = pscr.tile([128, BS], F32, tag="scr", name="e_t")
                nc.scalar.activation(e_t[:C, :], logits[:C, :], ACT.Exp)
                lse = pscr.tile([128, BS], F32, tag="scr", name="lse")
                for n in range(NB):
                    ps5 = ppsum.tile([128, 512], F32, tag="ps", name=f"ps5_{n}")
                    nc.tensor.matmul(ps5[:1, :], ones10[:C, :], e_t[:C, n * 512:(n + 1) * 512],
                                     start=True, stop=True)
                    nc.scalar.activation(lse[:1, n * 512:(n + 1) * 512], ps5[:1, :], ACT.Ln)
                lse10 = pscr.tile([128, BS], F32, tag="scr", name="lse10")
                nc.gpsimd.partition_broadcast(lse10[:C, :], lse[:1, :], channels=C)
                outs = plog.tile([16, BS], F32, tag="outs")
                nc.vector.tensor_tensor(outs[:C, :], logits[:C, :], lse10[:C, :], op=ALU.subtract)
                nc.sync.dma_start(out_d[:], outs[:C, :])

    nc.compile()
    return nc


def _pack_sign(W):
    """sign(W) [M*128, K*128] -> [128, M*K*128] with w[p, (m*K+k)*128+c] = sign(W)[m*128+c, k*128+p]."""
    S = np.where(np.asarray(W, np.float32) >= 0, np.float32(1), np.float32(-1))
    M, K = S.shape[0] // 128, S.shape[1] // 128
    A = S.reshape(M, 128, K, 128)  # [m, c, k, p]
    return np.ascontiguousarray(A.transpose(3, 0, 2, 1).reshape(128, M * K * 128))


def _prep_inputs(x, W1, b1, g1, bt1, W2, b2, g2, bt2, W3, b3, g3, bt3, W4, b4,
                 l1mode=None):
    """Host-side sharding + layout prep (pure layout/sign/lossless-split work)."""
    l1mode = l1mode or L1MODE

    def as32(a):
        return np.ascontiguousarray(np.asarray(a, dtype=np.float32))

    x = as32(x)
    s1 = _pack_sign(W1)
    shared = {
        "w1pk": s1.astype(np.float16),
        "w2pk": _pack_sign(W2).astype(ml_dtypes.float8_e4m3fn),
        "w3pk": _pack_sign(W3).astype(ml_dtypes.float8_e4m3fn),
    }
    if l1mode == "hi16lo8":
        shared["w1lopk"] = (s1 * 2.0 ** -11).astype(ml_dtypes.float8_e5m2)
    cvecs = (b1, g1, bt1, b2, g2, bt2, b3, g3, bt3)
    cpk = np.empty((128, KH * len(cvecs)), np.float32)
    for i, v in enumerate(cvecs):
        cpk[:, i * KH:(i + 1) * KH] = as32(v).reshape(KH, 128).T
    shared["cpk"] = cpk
    w4T = np.ascontiguousarray(as32(W4).T)          # [H, C]
    w4pk = np.empty((128, C * KH), np.float16)
    for k in range(KH):
        w4pk[:, k * C:(k + 1) * C] = w4T[k * 128:(k + 1) * 128, :].astype(np.float16)
    shared["w4pk"] = w4pk
    b4p = np.zeros((16, 1), np.float32)
    b4p[:C, 0] = as32(b4).reshape(-1)
    shared["c_b4"] = b4p

    in_maps = []
    for cidx in range(NCORES):
        xT = np.ascontiguousarray(x[cidx * BS:(cidx + 1) * BS].T)     # [D, BS]
        hi = xT.astype(np.float16)
        lo = xT - hi.astype(np.float32)
        m = dict(shared)
        m["xT_hi"] = hi
        if l1mode == "hi16lo8":
            m["xT_lo8"] = (lo * 2048.0).astype(ml_dtypes.float8_e5m2)
        else:
            m["xT_lo"] = lo.astype(np.float16)
        in_maps.append(m)
    return in_maps


def _fast_flags(inputs):
    """Mean-only BN boundaries are valid when beta==0 and gamma>0."""
    def ok(g, bt):
        g, bt = np.asarray(g), np.asarray(bt)
        return bool(not np.any(bt) and np.all(g > 0))

    return (ok(inputs["g1"], inputs["bt1"]), ok(inputs["g2"], inputs["bt2"]))


def kernel(**inputs) -> np.ndarray:
    from concourse.bass_utils import run_bass_kernel_spmd

    fast = _fast_flags(inputs)
    if _CACHE.get("key") != (fast, L1MODE):
        _CACHE["nc"] = _build(fast=fast)
        _CACHE["key"] = (fast, L1MODE)
    nc = _CACHE["nc"]
    in_maps = _prep_inputs(**inputs)
    res = run_bass_kernel_spmd(nc, in_maps, list(range(NCORES)))
    out = np.concatenate([res.results[c]["outT"].T for c in range(NCORES)], axis=0)
    return out.astype(np.float32)
